# revision 1
# baseline (speedup 1.0000x reference)
"""Trainium2 Bass kernel for nn_Classify_MLPPredictor (edge-parallel GNN inference).

Computes sigmoid(cat([h[src], h[dst]], -1) @ W + b) for E=1.6M edges over a
N=100k x 128 node table, on 8 NeuronCores.

Algorithm (per core, edges sharded 200k/core, h/W/b replicated):
  Phase 1: Pcat = h @ [Ws | Wd] + [0 | b]  -> two DRAM tables ps, pd [100k, 128]
           (factored form: avoids per-edge matmuls; each node row is reused
           ~16x by the gather phase).
  Phase 2: per 128-edge tile, indirect-DMA gather ps[src], pd[dst] into SBUF,
           add, sigmoid, write out rows.
"""

import os
import time

import numpy as np

import concourse.bass as bass
import concourse.bacc as bacc
import concourse.mybir as mybir
import concourse.tile as tile
from concourse.bass_utils import run_bass_kernel_spmd

N_CORES = 8
N_NODES = 100000
D = 128           # feature dim
C = 128           # classes
CC = 2 * C        # concatenated output cols of phase 1
E = 1600000
E_C = E // N_CORES            # 200000 edges per core

# phase 1 tiling
P1_CHUNK = 1024               # nodes per DMA chunk (8 matmul subtiles)

# phase 2 tiling
TILE_E = 128                  # edges per gather
TILES_PER_BLK = 32            # gathers fused into one add/sigmoid/store block
BLK_E = TILE_E * TILES_PER_BLK  # 4096

N_TILES = (E_C + TILE_E - 1) // TILE_E          # 1563 (last has 64 edges)
IDX_COLS = N_TILES                               # idx sbuf layout [128, N_TILES]

F32 = mybir.dt.float32
I32 = mybir.dt.int32

_CACHE = {}


def _build_program(repeat=1):
    nc = bacc.Bacc(None, target_bir_lowering=False)

    ht = nc.dram_tensor("ht", [D, N_NODES], F32, kind="ExternalInput")
    wcat = nc.dram_tensor("wcat", [D, CC], F32, kind="ExternalInput")
    bcat = nc.dram_tensor("bcat", [128, CC], F32, kind="ExternalInput")
    src_idx = nc.dram_tensor("src_idx", [128, IDX_COLS], I32, kind="ExternalInput")
    dst_idx = nc.dram_tensor("dst_idx", [128, IDX_COLS], I32, kind="ExternalInput")
    out = nc.dram_tensor("out", [E_C, C], F32, kind="ExternalOutput")

    ps = nc.dram_tensor("ps", [N_NODES, C], F32, kind="Internal")
    pd = nc.dram_tensor("pd", [N_NODES, C], F32, kind="Internal")

    with tile.TileContext(nc) as tc:
        with (
            tc.tile_pool(name="const", bufs=1) as cpool,
            tc.tile_pool(name="p1x", bufs=2) as xpool,
            tc.tile_pool(name="p1s", bufs=2) as spool,
            tc.tile_pool(name="psum", bufs=4, space="PSUM") as psum,
            tc.tile_pool(name="idx", bufs=1) as ipool,
            tc.tile_pool(name="g", bufs=2) as gpool,
            tc.tile_pool(name="o", bufs=2) as opool,
        ):
            wcat_t = cpool.tile([D, CC], F32)
            nc.sync.dma_start(out=wcat_t[:], in_=wcat[:])
            bcat_t = cpool.tile([128, CC], F32)
            nc.sync.dma_start(out=bcat_t[:], in_=bcat[:])

            # load all phase-2 indices up front (overlaps with phase 1)
            src_sb = ipool.tile([128, IDX_COLS], I32, tag="sidx")
            dst_sb = ipool.tile([128, IDX_COLS], I32, tag="didx")
            nc.sync.dma_start(out=src_sb[:], in_=src_idx[:])
            nc.sync.dma_start(out=dst_sb[:], in_=dst_idx[:])

            import contextlib

            rep_ctx = (
                tc.For_i(0, repeat, 1) if repeat > 1 else contextlib.nullcontext()
            )
            with rep_ctx:
                _emit_body(
                    nc, tc, xpool, spool, psum, gpool, opool,
                    ht, wcat_t, bcat_t, src_sb, dst_sb, ps, pd, out,
                )

    nc.compile()
    return nc


def _emit_body(nc, tc, xpool, spool, psum, gpool, opool,
               ht, wcat_t, bcat_t, src_sb, dst_sb, ps, pd, out):
    if True:
        if True:

            # ---------------- Phase 1: ps/pd = h @ [Ws|Wd] + [0|b] ----------------
            n0 = 0
            while n0 < N_NODES:
                nn = min(P1_CHUNK, N_NODES - n0)
                nsub = (nn + 127) // 128
                x = xpool.tile([D, P1_CHUNK], F32, tag="x")
                nc.sync.dma_start(out=x[:, :nn], in_=ht[:, n0 : n0 + nn])
                s = spool.tile([128, (P1_CHUNK // 128) * CC], F32, tag="s")
                for si in range(nsub):
                    m = min(128, nn - si * 128)
                    acc = psum.tile([128, CC], F32, tag="acc", space="PSUM")
                    nc.tensor.matmul(
                        acc[:m, :],
                        lhsT=x[:, si * 128 : si * 128 + m],
                        rhs=wcat_t[:],
                        start=True,
                        stop=True,
                    )
                    nc.vector.tensor_add(
                        out=s[:m, si * CC : (si + 1) * CC],
                        in0=acc[:m, :],
                        in1=bcat_t[:m, :],
                    )
                if nn == P1_CHUNK:
                    sv = s[:].rearrange("p (s q) -> p s q", s=nsub)
                    nc.sync.dma_start(
                        out=ps[n0 : n0 + nn, :].rearrange("(s p) c -> p s c", p=128),
                        in_=sv[:, :, 0:C],
                    )
                    nc.sync.dma_start(
                        out=pd[n0 : n0 + nn, :].rearrange("(s p) c -> p s c", p=128),
                        in_=sv[:, :, C:CC],
                    )
                else:
                    for si in range(nsub):
                        m = min(128, nn - si * 128)
                        r0 = n0 + si * 128
                        nc.sync.dma_start(
                            out=ps[r0 : r0 + m, :],
                            in_=s[:m, si * CC : si * CC + C],
                        )
                        nc.sync.dma_start(
                            out=pd[r0 : r0 + m, :],
                            in_=s[:m, si * CC + C : (si + 1) * CC],
                        )
                n0 += nn

            # ---------------- Phase 2: gather + add + sigmoid + store -------------
            t = 0
            while t < N_TILES:
                nt = min(TILES_PER_BLK, N_TILES - t)
                blk_w = nt * TILE_E
                gs = gpool.tile([128, BLK_E], F32, tag="gs")
                gd = gpool.tile([128, BLK_E], F32, tag="gd")
                for i in range(nt):
                    tt = t + i
                    pp = min(TILE_E, E_C - tt * TILE_E)
                    nc.gpsimd.indirect_dma_start(
                        out=gs[:pp, i * C : (i + 1) * C],
                        out_offset=None,
                        in_=ps[:, :],
                        in_offset=bass.IndirectOffsetOnAxis(
                            ap=src_sb[:pp, tt : tt + 1], axis=0
                        ),
                    )
                    nc.gpsimd.indirect_dma_start(
                        out=gd[:pp, i * C : (i + 1) * C],
                        out_offset=None,
                        in_=pd[:, :],
                        in_offset=bass.IndirectOffsetOnAxis(
                            ap=dst_sb[:pp, tt : tt + 1], axis=0
                        ),
                    )
                o = opool.tile([128, BLK_E], F32, tag="o")
                nc.vector.tensor_add(
                    out=gs[:, :blk_w], in0=gs[:, :blk_w], in1=gd[:, :blk_w]
                )
                nc.scalar.activation(
                    out=o[:, :blk_w],
                    in_=gs[:, :blk_w],
                    func=mybir.ActivationFunctionType.Sigmoid,
                )
                # full 128-row tiles in this block
                nfull = nt if (t + nt) * TILE_E <= E_C else nt - 1
                if nfull > 0:
                    r0 = t * TILE_E
                    nc.sync.dma_start(
                        out=out[r0 : r0 + nfull * 128, :].rearrange(
                            "(i p) c -> p i c", p=128
                        ),
                        in_=o[:, : nfull * C].rearrange("p (i c) -> p i c", c=C),
                    )
                if nfull < nt:  # trailing partial tile (64 edges)
                    i = nt - 1
                    tt = t + i
                    pp = E_C - tt * TILE_E
                    nc.sync.dma_start(
                        out=out[tt * TILE_E : tt * TILE_E + pp, :],
                        in_=o[:pp, i * C : i * C + C],
                    )
                t += nt


def _prep_inputs(h, src, dst, W, b):
    h = np.asarray(h, dtype=np.float32)
    src = np.asarray(src)
    dst = np.asarray(dst)
    W = np.asarray(W, dtype=np.float32)
    b = np.asarray(b, dtype=np.float32)

    ht = np.ascontiguousarray(h.T)                      # [128, 100000]
    wcat = np.ascontiguousarray(
        np.concatenate([W[:D], W[D:]], axis=1)          # [128, 256]
    )
    bcat = np.ascontiguousarray(
        np.tile(np.concatenate([np.zeros(C, np.float32), b])[None, :], (128, 1))
    )

    in_maps = []
    for c in range(N_CORES):
        s = src[c * E_C : (c + 1) * E_C].astype(np.int32)
        d = dst[c * E_C : (c + 1) * E_C].astype(np.int32)
        pad = N_TILES * TILE_E - E_C
        if pad:
            s = np.concatenate([s, np.zeros(pad, np.int32)])
            d = np.concatenate([d, np.zeros(pad, np.int32)])
        # [128, N_TILES]: element [p, t] = index of edge t*128 + p
        s2 = np.ascontiguousarray(s.reshape(N_TILES, 128).T)
        d2 = np.ascontiguousarray(d.reshape(N_TILES, 128).T)
        in_maps.append(
            {
                "ht": ht,
                "wcat": wcat,
                "bcat": bcat,
                "src_idx": s2,
                "dst_idx": d2,
            }
        )
    return in_maps


def kernel(h, src, dst, W, b):
    if "nc" not in _CACHE:
        t0 = time.time()
        _CACHE["nc"] = _build_program()
        if os.environ.get("KERNEL_VERBOSE"):
            print(f"[kernel] build+compile: {time.time() - t0:.1f}s")
    nc = _CACHE["nc"]
    in_maps = _prep_inputs(h, src, dst, W, b)
    res = run_bass_kernel_spmd(nc, in_maps, core_ids=list(range(N_CORES)))
    outs = [res.results[c]["out"] for c in range(N_CORES)]
    return np.concatenate(outs, axis=0)



# revision 5
# speedup vs baseline: 3.2710x; 3.2710x over previous
"""Trainium2 Bass kernel for nn_Classify_MLPPredictor (edge-parallel GNN inference).

Computes sigmoid(h[src] @ Ws + h[dst] @ Wd + b) for E=1.6M edges over a
N=100k x 128 node table, on 8 NeuronCores.

Sharding: 8 cells = 2 src-halves x 4 dst-quarters. Core c owns edges with
src in half (c//4) and dst in quarter (c%4)  (~200k edges/core).

Per-core algorithm (all descriptor-heavy random access eliminated or moved
to SBUF-resident bulk gathers):
  Phase 1: pd quarter-table (25088 rows x 128 fp16) = h[q] @ Wd + b, built
           with 196 matmuls from sequential ht reads, kept RESIDENT IN SBUF
           in dma_gather layout (row r -> partition r%128, col-block r//128).
  Phase 2: edges sorted by src, greedy-packed into N_W windows (whole nodes,
           <=128 nodes and <=512 edge slots each):
             ys: ps_w = htw_w @ Ws (on-the-fly, PSUM->SBUF fp16), expansion
                 to edges via one-hot "staircase" matmul M[node, e] built from
                 per-node start/end vectors (3 DVE compare ops) -> ys [C,512].
             yd: bulk SBUF->SBUF dma_gather (transpose mode, int16 idx) of
                 pd rows for 4096 edges/op -> [C, 4096] fp16.
             out: DVE add + ACT sigmoid -> fp16, staged and written as
                 outT [128(C), N_W*512] (host transposes/unpermutes/casts).
"""

import os
import time

import numpy as np

import concourse.bass as bass
import concourse.bacc as bacc
import concourse.mybir as mybir
import concourse.tile as tile
from concourse.bass_utils import run_bass_kernel_spmd

N_CORES = 8
N_NODES = 100000
D = 128
C = 128
E = 1600000

HALF = 50000      # src shard size (2 halves)
QUART = 25000     # dst shard size (4 quarters)
QPAD = 25088      # padded dst table rows (196*128)
QCHUNKS = QPAD // 128

W_E = 512                 # edge slots per window
GB = 4096                 # gather batch (8 windows)
W_PER_B = GB // W_E

F32 = mybir.dt.float32
F16 = mybir.dt.float16
I16 = mybir.dt.int16

_CACHE = {}


def _build_program(n_w, repeat=1):
    """Static program parameterized only by the window count n_w."""
    import contextlib

    assert n_w % W_PER_B == 0
    n_b = n_w // W_PER_B          # gather batches
    epad = n_w * W_E              # padded edges per core

    nc = bacc.Bacc(None, target_bir_lowering=False)

    htq = nc.dram_tensor("htq", [D, QPAD], F16, kind="ExternalInput")
    htw = nc.dram_tensor("htw", [D, n_w * 128], F16, kind="ExternalInput")
    ws = nc.dram_tensor("ws", [D, C], F16, kind="ExternalInput")
    wd = nc.dram_tensor("wd", [D, C], F16, kind="ExternalInput")
    bcat = nc.dram_tensor("bcat", [128, C], F32, kind="ExternalInput")
    sv = nc.dram_tensor("sv", [128, n_w], F32, kind="ExternalInput")
    ev = nc.dram_tensor("ev", [128, n_w], F32, kind="ExternalInput")
    iot = nc.dram_tensor("iot", [128, W_E], F32, kind="ExternalInput")
    didx = nc.dram_tensor("didx", [128, epad // 16], I16, kind="ExternalInput")
    outT = nc.dram_tensor("outT", [128, epad], F16, kind="ExternalOutput")

    with tile.TileContext(nc) as tc:
        with (
            tc.tile_pool(name="const", bufs=1) as cpool,
            tc.tile_pool(name="tab", bufs=1) as tabpool,
            tc.tile_pool(name="x", bufs=3) as xpool,
            tc.tile_pool(name="psw", bufs=3) as pswpool,
            tc.tile_pool(name="m", bufs=3) as mpool,
            tc.tile_pool(name="yd", bufs=3) as ydpool,
            tc.tile_pool(name="o", bufs=3) as opool,
            tc.tile_pool(name="idx", bufs=3) as ipool,
            tc.tile_pool(name="psum", bufs=2, space="PSUM") as psum,
            tc.tile_pool(name="psy", bufs=4, space="PSUM") as psy,
        ):
            ws_t = cpool.tile([D, C], F16)
            nc.sync.dma_start(out=ws_t[:], in_=ws[:])
            wd_t = cpool.tile([D, C], F16)
            nc.sync.dma_start(out=wd_t[:], in_=wd[:])
            bc_t = cpool.tile([128, C], F32)
            nc.sync.dma_start(out=bc_t[:], in_=bcat[:])
            sv_t = cpool.tile([128, n_w], F32)
            nc.sync.dma_start(out=sv_t[:], in_=sv[:])
            ev_t = cpool.tile([128, n_w], F32)
            nc.sync.dma_start(out=ev_t[:], in_=ev[:])
            io_t = cpool.tile([128, W_E], F32)
            nc.sync.dma_start(out=io_t[:], in_=iot[:])

            pdtab = tabpool.tile([128, QCHUNKS * C], F16)

            rep = tc.For_i(0, repeat, 1) if repeat > 1 else contextlib.nullcontext()
            with rep:
                # ---- Phase 1: pd quarter table, SBUF-resident ----
                for k in range(QCHUNKS):
                    x = xpool.tile([D, 128], F16, tag="x1")
                    nc.sync.dma_start(out=x[:], in_=htq[:, k * 128 : (k + 1) * 128])
                    acc = psum.tile([128, C], F32, tag="pd", space="PSUM")
                    nc.tensor.matmul(
                        acc[:], lhsT=x[:], rhs=wd_t[:], start=True, stop=True
                    )
                    nc.vector.tensor_add(
                        out=pdtab[:, k * C : (k + 1) * C], in0=acc[:], in1=bc_t[:]
                    )

                # ---- Phase 2 ----
                for b in range(n_b):
                    ids = ipool.tile([128, GB // 16], I16, tag="idx")
                    nc.sync.dma_start(
                        out=ids[:], in_=didx[:, b * (GB // 16) : (b + 1) * (GB // 16)]
                    )
                    yd = ydpool.tile([128, GB], F16, tag="yd")
                    nc.gpsimd.dma_gather(
                        yd[:].rearrange("p (u e) -> p u e", u=1),
                        pdtab[:],
                        ids[:],
                        GB,
                        GB,
                        C,
                        transpose=True,
                        single_packet=False,
                        sbuf_tokens_per_rank=128,
                        sbuf_free_dim_per_rank=C * 2,
                    )
                    ost = opool.tile([128, GB], F16, tag="o")
                    for j in range(W_PER_B):
                        w = b * W_PER_B + j
                        x = xpool.tile([D, 128], F16, tag="x2")
                        nc.sync.dma_start(
                            out=x[:], in_=htw[:, w * 128 : (w + 1) * 128]
                        )
                        pacc = psum.tile([128, C], F32, tag="ps", space="PSUM")
                        nc.tensor.matmul(
                            pacc[:], lhsT=x[:], rhs=ws_t[:], start=True, stop=True
                        )
                        psw = pswpool.tile([128, C], F16, tag="psw")
                        nc.vector.tensor_copy(out=psw[:], in_=pacc[:])
                        # staircase one-hot M[node, e] = (e >= S_n) & (e < E_n)
                        m = mpool.tile([128, W_E], F16, tag="m")
                        ge = mpool.tile([128, W_E], F16, tag="ge")
                        nc.vector.tensor_scalar(
                            out=ge[:],
                            in0=io_t[:],
                            scalar1=sv_t[:, w : w + 1],
                            scalar2=None,
                            op0=mybir.AluOpType.is_ge,
                        )
                        nc.vector.scalar_tensor_tensor(
                            out=m[:],
                            in0=io_t[:],
                            scalar=ev_t[:, w : w + 1],
                            in1=ge[:],
                            op0=mybir.AluOpType.is_lt,
                            op1=mybir.AluOpType.mult,
                        )
                        ys = psy.tile([128, W_E], F32, tag="ys", space="PSUM")
                        nc.tensor.matmul(
                            ys[:], lhsT=psw[:], rhs=m[:], start=True, stop=True
                        )
                        s = opool.tile([128, W_E], F32, tag="s")
                        nc.vector.tensor_add(
                            out=s[:], in0=ys[:], in1=yd[:, j * W_E : (j + 1) * W_E]
                        )
                        nc.scalar.activation(
                            out=ost[:, j * W_E : (j + 1) * W_E],
                            in_=s[:],
                            func=mybir.ActivationFunctionType.Sigmoid,
                        )
                    nc.sync.dma_start(
                        out=outT[:, b * GB : (b + 1) * GB], in_=ost[:]
                    )

    nc.compile()
    return nc


def _pack_windows(s, deg_half):
    """Greedy pack whole src nodes into windows (<=128 nodes, <=W_E edges).

    s: per-edge src ids (sorted), relative to half base.
    deg_half: degree of every node in this half (len HALF).
    Returns list of (node_lo, node_hi, edge_lo, edge_hi).
    """
    wins = []
    n = 0
    e = 0
    n_nodes = len(deg_half)
    while n < n_nodes:
        cnt = 0
        n0 = n
        while n < n_nodes and (n - n0) < 128:
            d = deg_half[n]
            if cnt + d > W_E:
                break
            cnt += d
            n += 1
        if n == n0:
            raise RuntimeError("single node exceeds window capacity")
        wins.append((n0, n, e, e + cnt))
        e += cnt
    return wins


def _prep_inputs(h, src, dst, W, b):
    h = np.asarray(h, dtype=np.float32)
    src = np.asarray(src).astype(np.int64)
    dst = np.asarray(dst).astype(np.int64)
    W = np.asarray(W, dtype=np.float32)
    b = np.asarray(b, dtype=np.float32)

    ws = np.ascontiguousarray(W[:D]).astype(np.float16)   # [D, C]
    wd = np.ascontiguousarray(W[D:]).astype(np.float16)
    bcat = np.tile(b[None, :], (128, 1)).astype(np.float32)
    iot = np.tile(np.arange(W_E, dtype=np.float32)[None, :], (128, 1))

    h16 = h.astype(np.float16)

    cores = []
    for c in range(N_CORES):
        sh, dq = c // 4, c % 4
        mask = (src // HALF == sh) & (dst // QUART == dq)
        e_ids = np.flatnonzero(mask)
        s = (src[e_ids] - sh * HALF).astype(np.int64)
        d = (dst[e_ids] - dq * QUART).astype(np.int16)
        order = np.argsort(s, kind="stable")
        s, d, e_ids = s[order], d[order], e_ids[order]
        deg = np.bincount(s, minlength=HALF)
        wins = _pack_windows(s, deg)
        cores.append(dict(sh=sh, dq=dq, s=s, d=d, e_ids=e_ids, deg=deg, wins=wins))

    n_w_val = max(len(ci["wins"]) for ci in cores)
    n_w_val = ((n_w_val + W_PER_B - 1) // W_PER_B) * W_PER_B
    epad = n_w_val * W_E

    in_maps = []
    hostmeta = []
    for c in range(N_CORES):
        ci = cores[c]
        sh, dq, wins, deg = ci["sh"], ci["dq"], ci["wins"], ci["deg"]

        # htq: dst quarter features [D, QPAD]
        htq = np.zeros((D, QPAD), np.float16)
        htq[:, :QUART] = h16[dq * QUART : (dq + 1) * QUART].T

        # per-window data
        htw = np.zeros((D, n_w_val * 128), np.float16)
        sv = np.zeros((128, n_w_val), np.float32)
        ev = np.zeros((128, n_w_val), np.float32)
        didx_flat = np.zeros(epad, np.int16)
        pos_of_edge = np.zeros(len(ci["s"]), np.int64)

        for w, (n0, n1, e0, e1) in enumerate(wins):
            nn = n1 - n0
            nodes = np.arange(n0, n1)
            htw[:, w * 128 : w * 128 + nn] = h16[sh * HALF + n0 : sh * HALF + n1].T
            dgs = deg[n0:n1]
            ends = np.cumsum(dgs)
            starts = ends - dgs
            sv[:nn, w] = starts.astype(np.float32)
            ev[:nn, w] = ends.astype(np.float32)
            cnt = e1 - e0
            didx_flat[w * W_E : w * W_E + cnt] = ci["d"][e0:e1]
            pos_of_edge[e0:e1] = w * W_E + np.arange(cnt)

        # wrap idx: element i -> partition i%16, col i//16; replicate 8x
        didx = np.tile(
            didx_flat.reshape(epad // 16, 16).T, (8, 1)
        ).copy()

        in_maps.append(
            {
                "htq": htq,
                "htw": htw,
                "ws": ws,
                "wd": wd,
                "bcat": bcat,
                "sv": sv,
                "ev": ev,
                "iot": iot,
                "didx": didx,
            }
        )
        hostmeta.append((ci["e_ids"], pos_of_edge))
    return n_w_val, in_maps, hostmeta


def kernel(h, src, dst, W, b):
    n_w_val, in_maps, hostmeta = _prep_inputs(h, src, dst, W, b)
    key = ("nc", n_w_val)
    if key not in _CACHE:
        t0 = time.time()
        _CACHE[key] = _build_program(n_w_val)
        if os.environ.get("KERNEL_VERBOSE"):
            print(f"[kernel] build+compile: {time.time() - t0:.1f}s")
    nc = _CACHE[key]
    res = run_bass_kernel_spmd(nc, in_maps, core_ids=list(range(N_CORES)))
    out = np.empty((E, C), np.float32)
    for c in range(N_CORES):
        e_ids, pos = hostmeta[c]
        oT = res.results[c]["outT"]  # [128, epad] fp16
        out[e_ids] = oT.T[pos].astype(np.float32)
    return out
